# revision 4
# baseline (speedup 1.0000x reference)
"""CSNN (spiking conv net with WTA dynamics) on 8 Trainium2 NeuronCores.

The WTA event scan vectorizes across output columns (SBUF partitions);
output channels ride the free dim. Host pre-sorts each column's events by
spike time and pre-gathers weight rows into a (P, S*F) stream (zero rows pad
short columns; they add 0, never fire, and are exact no-ops).

Device per event step (the serial critical path is 3 instructions):
  pass1 (custom DVE op): pre = potU * sel(m_prev > TH, rt_prev, 1) + w_s,
                         with a fused max-reduction -> m_s.
      potU holds the *unnormalized* committed state (the winner-zeroed exp
      values when the previous step fired, the raw potential otherwise);
      multiplying by rt = 1/Z here reproduces fl(e_i * rt) + w bit-exactly,
      so normalization never occupies its own instruction.
  ACT:  e = Exp(pre), accumulator -> Z (sum includes the winner, as in the
        reference softmax).
  pass2 (custom DVE op): potU' = sel(m > TH, (pre != m) * e, pre)
      i.e. on fire commit the unnormalized exps with every argmax tie zeroed
      (exp > 0 elsewhere), else keep pre.
  off-path: DVE reciprocal (1/Z, ~9ns), GPSIMD records dd = (potU' == 0)
      (the winner one-hot) into a streamed uint8 buffer.

The device returns per-step column maxima (m) and winner one-hots (dd); the
host reconstructs spike times spk[p,f] = max_s fire[p,s]*dd[p,s,f]*T[p,s]
(event times ascend, so max == last, matching the reference overwrite).
Layers are separate launches; columns are sharded 8 ways.
"""
import numpy as np

import concourse.bacc as bacc
import concourse.mybir as mybir
from concourse.tile import TileContext
from concourse import bass_utils
from concourse import dve_ops
from concourse.dve_spec import (Spec, Src0, Src1, C0, C1, C2, Zero, One, MaxNeg,
                                select, ne, maxx, lower as dve_lower, _has_src1,
                                AluOp as DALU)
from concourse.dve_uop import DveOpSpec

F32 = np.float32
BF32 = mybir.dt.float32
Exp = mybir.ActivationFunctionType.Exp
ALU = mybir.AluOpType

LAYERS = [
    dict(cout=30, k=5, pad=2, th=2.4),
    dict(cout=100, k=3, pad=1, th=1.0),
    dict(cout=200, k=3, pad=1, th=1.0),
]
N_CORES = 8


# ------------------------------------------------------- custom DVE ops

def _register_dve(name, spec):
    for op in dve_ops.OPS:
        if op.name == name:
            return op
    row = dve_ops._CUSTOM_DVE_ROW_BASE + len(dve_ops.OPS)
    assert row < 0x20, "custom-DVE row field overflow"
    shas = {}
    for ver in ("v3",):  # TRN2
        uops = dve_lower(spec, ver=ver)
        shas[ver] = DveOpSpec(name=name, opcode=row, uops=uops,
                              rd1_en=_has_src1(spec)).sha(ver)
    op = dve_ops.DveOp(name, spec, subdim=False, uops_sha=shas)
    dve_ops.OPS.append(op)
    dve_ops._SUB_OPCODE_FOR_NAME[name] = row
    return op


def _ref_p1(in0, in1, s0, s1, imm2):
    b = (in0 * np.where(s1 > imm2, s0, 1.0) + in1).astype(np.float32)
    return b, np.maximum(b.reshape(b.shape[0], -1).max(-1, keepdims=True), 0.0)


# pre = potU * (m_prev > TH ? rt_prev : 1) + w   ; accum_out = max(pre)
# (maxx(Src0, MaxNeg) is an identity pad so the hoisted select is read at
# stage >= 1 -- latch-init can't build it by stage 0.)
WTA_P1 = _register_dve("WTA_P1", Spec(
    body=maxx(Src0, MaxNeg) * select(C1 > C2, C0, One) + Src1,
    accum=DALU.MAX, accum_init=Zero, reference=_ref_p1))

# potU' = (m > TH) ? (pre != m) * e : pre
WTA_P2 = _register_dve("WTA_P2", Spec(
    body=select(C0 > C2, ne(Src1, C0) * Src0, Src1),
    reference=lambda in0, in1, s0, s1, imm2: np.where(
        s0 > imm2, (in1 != s0).astype(np.float32) * in0, in1)))


# ---------------------------------------------------------------- host side

def _unfold_buggy(x, k):
    C, H, W = x.shape
    oh, ow = H - k + 1, W - k + 1
    ih = np.arange(oh)[:, None] + np.arange(k)[None, :]
    iw = np.arange(ow)[:, None] + np.arange(k)[None, :]
    p = x[:, ih[:, None, :, None], iw[None, :, None, :]]
    unf = p.transpose(0, 3, 4, 1, 2).reshape(C * k * k, oh * ow)
    return unf.reshape(C, oh * ow, k * k), oh, ow


def _build_events(spk_in, weights, pad):
    cout, cin, k, _ = weights.shape
    x = np.pad(spk_in.astype(F32), ((0, 0), (pad, pad), (pad, pad)))
    x_trans, oh, ow = _unfold_buggy(x, k)
    L, k2 = oh * ow, k * k
    w_r = np.ascontiguousarray(weights.reshape(cout, cin * k2).T.astype(F32))
    tv = x_trans.transpose(1, 0, 2).reshape(L, cin * k2)
    order = np.argsort(np.where(tv != 0, tv, np.inf), axis=1, kind='stable')
    nvalid = (tv != 0).sum(axis=1)
    S = max(1, int(nvalid.max()))
    order = order[:, :S]
    tsort = np.take_along_axis(tv, order, axis=1)
    valid = np.arange(S)[None, :] < nvalid[:, None]
    W_seq = w_r[order]
    W_seq[~valid] = 0.0
    T_seq = np.where(valid, tsort, 0.0).astype(F32)
    return np.ascontiguousarray(W_seq), T_seq, S, oh, ow


def _shard(W_seq):
    L, S, F = W_seq.shape
    Pc = (L + N_CORES - 1) // N_CORES
    Wp = np.zeros((Pc * N_CORES, S, F), F32)
    Wp[:L] = W_seq
    Ws = [np.ascontiguousarray(Wp[i * Pc:(i + 1) * Pc].reshape(Pc, S * F))
          for i in range(N_CORES)]
    return Ws, Pc


def _max_pool2(x):
    C, H, W = x.shape
    oh, ow = H // 2, W // 2
    return x[:, :oh * 2, :ow * 2].reshape(C, oh, 2, ow, 2).max(axis=(2, 4))


# -------------------------------------------------------------- device side

def _build_layer(P, F, S, TH, CS=None):
    if CS is None:
        # 3 W bufs (f32) + 3 dd bufs (u8) per chunk step: 15*F bytes/partition
        CS = max(1, min(S, (160 * 1024) // (15 * F)))
    nc = bacc.Bacc("TRN2", target_bir_lowering=False, debug=False)
    Wd = nc.dram_tensor("W", (P, S * F), BF32, kind="ExternalInput")
    Md = nc.dram_tensor("m", (P, S), BF32, kind="ExternalOutput")
    Dd = nc.dram_tensor("dd", (P, S * F), mybir.dt.uint8, kind="ExternalOutput")

    with TileContext(nc) as tc:
        with (
            tc.tile_pool(name="state", bufs=1) as st,
            tc.tile_pool(name="wpool", bufs=3) as wp,
            tc.tile_pool(name="dpool", bufs=3) as dp,
        ):
            A = st.tile([P, F], BF32)      # pre-commit potential
            B = st.tile([P, F], BF32)      # committed (unnormalized) state
            e = st.tile([P, F], BF32)
            mrec = st.tile([P, S], BF32)
            mz = st.tile([P, 1], BF32)     # m_prev for step 0
            Zt = st.tile([P, 2], BF32)     # parity-buffered Z
            rt = st.tile([P, 2], BF32)     # parity-buffered 1/Z

            nc.vector.memset(B[:], 0.0)
            nc.vector.memset(mz[:], 0.0)
            nc.vector.memset(rt[:], 1.0)
            nc.vector.memset(Zt[:], 1.0)

            for ci in range((S + CS - 1) // CS):
                s0, s1 = ci * CS, min(S, ci * CS + CS)
                wt = wp.tile([P, (s1 - s0) * F], BF32, tag="w")
                dt = dp.tile([P, (s1 - s0) * F], mybir.dt.uint8, tag="d")
                nc.sync.dma_start(wt[:], Wd[:, s0 * F:s1 * F])
                for s in range(s0, s1):
                    q, j = s & 1, s - s0
                    mprev = mz[:, 0:1] if s == 0 else mrec[:, s - 1:s]
                    nc.vector._custom_dve(
                        WTA_P1, out=A[:], in0=B[:], in1=wt[:, j * F:(j + 1) * F],
                        s0=rt[:, 1 - q:2 - q], s1=mprev, imm2=TH,
                        accum_out=mrec[:, s:s + 1])
                    nc.scalar.activation(e[:], A[:], Exp, accum_out=Zt[:, q:q + 1])
                    nc.vector._custom_dve(
                        WTA_P2, out=B[:], in0=e[:], in1=A[:],
                        s0=mrec[:, s:s + 1], imm2=TH)
                    nc.vector.reciprocal(rt[:, q:q + 1], Zt[:, q:q + 1])
                    nc.gpsimd.tensor_scalar(dt[:, j * F:(j + 1) * F], B[:],
                                            0.0, None, ALU.is_equal)
                nc.sync.dma_start(Dd[:, s0 * F:s1 * F], dt[:])
            nc.sync.dma_start(Md[:], mrec[:])
    nc.finalize()
    return nc


_LAYER_RESULTS_NS = []


def _run_layer(Ws, F, TH, S, Pc, trace=False):
    nc = _build_layer(Pc, F, S, TH)
    in_maps = [{"W": w} for w in Ws]
    res = bass_utils.run_bass_kernel_spmd(
        nc, in_maps, core_ids=list(range(N_CORES)), trace=trace)
    _LAYER_RESULTS_NS.append(res.exec_time_ns)
    return res.results


def kernel(x, w1, w2, w3, _trace=False):
    _LAYER_RESULTS_NS.clear()
    s = np.asarray(x, F32)
    for w, cfg in zip((w1, w2, w3), LAYERS):
        W_seq, T_seq, S, oh, ow = _build_events(s, np.asarray(w, F32), cfg['pad'])
        Ws, Pc = _shard(W_seq)
        F, TH = cfg['cout'], cfg['th']
        outs = _run_layer(Ws, F, TH, S, Pc, trace=_trace)
        L = oh * ow
        mrec = np.concatenate([r["m"] for r in outs], axis=0)[:L]        # (L, S)
        dd = np.concatenate([r["dd"] for r in outs], axis=0)[:L]         # (L, S*F)
        tg = np.where(mrec > TH, T_seq, 0.0).astype(F32)                 # (L, S)
        spk = (dd.reshape(L, S, F).astype(F32) * tg[:, :, None]).max(axis=1)
        s = _max_pool2(np.ascontiguousarray(spk.T.reshape(F, oh, ow)))
    return np.ascontiguousarray(s)


# revision 5
# speedup vs baseline: 2.1724x; 2.1724x over previous
"""CSNN (spiking conv net with WTA dynamics) on 8 Trainium2 NeuronCores.

The WTA event scan vectorizes across output columns (SBUF partitions);
output channels ride the free dim. Host pre-sorts each column's events by
spike time and pre-gathers weight rows into a (P, S*F) stream (zero rows pad
short columns; they add 0, never fire, and are exact no-ops).

Device per event step (the serial critical path is 3 instructions):
  pass1 (custom DVE op): pre = potU * sel(m_prev > TH, rt_prev, 1) + w_s,
                         with a fused max-reduction -> m_s.
      potU holds the *unnormalized* committed state (the winner-zeroed exp
      values when the previous step fired, the raw potential otherwise);
      multiplying by rt = 1/Z here reproduces fl(e_i * rt) + w bit-exactly,
      so normalization never occupies its own instruction.
  ACT:  e = Exp(pre), accumulator -> Z (sum includes the winner, as in the
        reference softmax).
  pass2 (custom DVE op): potU' = sel(m > TH, (pre != m) * e, pre)
      i.e. on fire commit the unnormalized exps with every argmax tie zeroed
      (exp > 0 elsewhere), else keep pre.
  off-path: DVE reciprocal (1/Z, ~9ns), GPSIMD records dd = (potU' == 0)
      (the winner one-hot) into a streamed uint8 buffer.

The device returns per-step column maxima (m) and winner one-hots (dd); the
host reconstructs spike times spk[p,f] = max_s fire[p,s]*dd[p,s,f]*T[p,s]
(event times ascend, so max == last, matching the reference overwrite).
Layers are separate launches; columns are sharded 8 ways.
"""
import numpy as np

import concourse.bacc as bacc
import concourse.mybir as mybir
from concourse.tile import TileContext
from concourse import bass_utils
from concourse import dve_ops
from concourse.dve_spec import (Spec, Src0, Src1, C0, C1, C2, Zero, One, MaxNeg,
                                select, ne, maxx, lower as dve_lower, _has_src1,
                                AluOp as DALU)
from concourse.dve_uop import DveOpSpec

F32 = np.float32
BF32 = mybir.dt.float32
Exp = mybir.ActivationFunctionType.Exp
ALU = mybir.AluOpType

LAYERS = [
    dict(cout=30, k=5, pad=2, th=2.4),
    dict(cout=100, k=3, pad=1, th=1.0),
    dict(cout=200, k=3, pad=1, th=1.0),
]
N_CORES = 8


# ------------------------------------------------------- custom DVE ops

def _register_dve(name, spec):
    for op in dve_ops.OPS:
        if op.name == name:
            return op
    row = dve_ops._CUSTOM_DVE_ROW_BASE + len(dve_ops.OPS)
    assert row < 0x20, "custom-DVE row field overflow"
    shas = {}
    for ver in ("v3",):  # TRN2
        uops = dve_lower(spec, ver=ver)
        shas[ver] = DveOpSpec(name=name, opcode=row, uops=uops,
                              rd1_en=_has_src1(spec)).sha(ver)
    op = dve_ops.DveOp(name, spec, subdim=False, uops_sha=shas)
    dve_ops.OPS.append(op)
    dve_ops._SUB_OPCODE_FOR_NAME[name] = row
    return op


def _ref_p1(in0, in1, s0, s1, imm2):
    b = (in0 * np.where(s1 > imm2, s0, 1.0) + in1).astype(np.float32)
    return b, np.maximum(b.reshape(b.shape[0], -1).max(-1, keepdims=True), 0.0)


# pre = potU * (m_prev > TH ? rt_prev : 1) + w   ; accum_out = max(pre)
# (maxx(Src0, MaxNeg) is an identity pad so the hoisted select is read at
# stage >= 1 -- latch-init can't build it by stage 0.)
WTA_P1 = _register_dve("WTA_P1", Spec(
    body=maxx(Src0, MaxNeg) * select(C1 > C2, C0, One) + Src1,
    accum=DALU.MAX, accum_init=Zero, reference=_ref_p1))

# potU' = (m > TH) ? (pre != m) * e : pre
WTA_P2 = _register_dve("WTA_P2", Spec(
    body=select(C0 > C2, ne(Src1, C0) * Src0, Src1),
    reference=lambda in0, in1, s0, s1, imm2: np.where(
        s0 > imm2, (in1 != s0).astype(np.float32) * in0, in1)))


# ---------------------------------------------------------------- host side

def _unfold_buggy(x, k):
    C, H, W = x.shape
    oh, ow = H - k + 1, W - k + 1
    ih = np.arange(oh)[:, None] + np.arange(k)[None, :]
    iw = np.arange(ow)[:, None] + np.arange(k)[None, :]
    p = x[:, ih[:, None, :, None], iw[None, :, None, :]]
    unf = p.transpose(0, 3, 4, 1, 2).reshape(C * k * k, oh * ow)
    return unf.reshape(C, oh * ow, k * k), oh, ow


def _build_events(spk_in, weights, pad):
    cout, cin, k, _ = weights.shape
    x = np.pad(spk_in.astype(F32), ((0, 0), (pad, pad), (pad, pad)))
    x_trans, oh, ow = _unfold_buggy(x, k)
    L, k2 = oh * ow, k * k
    w_r = np.ascontiguousarray(weights.reshape(cout, cin * k2).T.astype(F32))
    tv = x_trans.transpose(1, 0, 2).reshape(L, cin * k2)
    order = np.argsort(np.where(tv != 0, tv, np.inf), axis=1, kind='stable')
    nvalid = (tv != 0).sum(axis=1)
    S = max(1, int(nvalid.max()))
    order = order[:, :S]
    tsort = np.take_along_axis(tv, order, axis=1)
    valid = np.arange(S)[None, :] < nvalid[:, None]
    W_seq = w_r[order]
    W_seq[~valid] = 0.0
    T_seq = np.where(valid, tsort, 0.0).astype(F32)
    return np.ascontiguousarray(W_seq), T_seq, S, oh, ow


def _shard(W_seq):
    L, S, F = W_seq.shape
    Pc = (L + N_CORES - 1) // N_CORES
    Wp = np.zeros((Pc * N_CORES, S, F), F32)
    Wp[:L] = W_seq
    Ws = [np.ascontiguousarray(Wp[i * Pc:(i + 1) * Pc].reshape(Pc, S * F))
          for i in range(N_CORES)]
    return Ws, Pc


def _max_pool2(x):
    C, H, W = x.shape
    oh, ow = H // 2, W // 2
    return x[:, :oh * 2, :ow * 2].reshape(C, oh, 2, ow, 2).max(axis=(2, 4))


# -------------------------------------------------------------- device side

def _build_layer(P, F, S, TH, CS=None):
    if CS is None:
        # 3 W bufs (f32) + 3 dd bufs (u8) per chunk step: 15*F bytes/partition
        CS = max(1, min(S, (160 * 1024) // (15 * F)))
    nc = bacc.Bacc("TRN2", target_bir_lowering=False, debug=False)
    Wd = nc.dram_tensor("W", (P, S * F), BF32, kind="ExternalInput")
    Md = nc.dram_tensor("m", (P, S), BF32, kind="ExternalOutput")
    Dd = nc.dram_tensor("dd", (P, S * F), mybir.dt.uint8, kind="ExternalOutput")

    with TileContext(nc) as tc:
        with (
            tc.tile_pool(name="state", bufs=1) as st,
            tc.tile_pool(name="wpool", bufs=3) as wp,
            tc.tile_pool(name="dpool", bufs=3) as dp,
        ):
            A = st.tile([P, F], BF32)      # pre-commit potential
            B = st.tile([P, F], BF32)      # committed (unnormalized) state
            e = st.tile([P, F], BF32)
            mrec = st.tile([P, S], BF32)
            mz = st.tile([P, 1], BF32)     # m_prev for step 0
            Zt = st.tile([P, 2], BF32)     # parity-buffered Z
            rt = st.tile([P, 2], BF32)     # parity-buffered 1/Z

            nc.vector.memset(B[:], 0.0)
            nc.vector.memset(mz[:], 0.0)
            nc.vector.memset(rt[:], 1.0)
            nc.vector.memset(Zt[:], 1.0)

            for ci in range((S + CS - 1) // CS):
                s0, s1 = ci * CS, min(S, ci * CS + CS)
                wt = wp.tile([P, (s1 - s0) * F], BF32, tag="w")
                dt = dp.tile([P, (s1 - s0) * F], mybir.dt.uint8, tag="d")
                nc.sync.dma_start(wt[:], Wd[:, s0 * F:s1 * F])
                for s in range(s0, s1):
                    q, j = s & 1, s - s0
                    mprev = mz[:, 0:1] if s == 0 else mrec[:, s - 1:s]
                    nc.vector._custom_dve(
                        WTA_P1, out=A[:], in0=B[:], in1=wt[:, j * F:(j + 1) * F],
                        s0=rt[:, 1 - q:2 - q], s1=mprev, imm2=TH,
                        accum_out=mrec[:, s:s + 1])
                    nc.scalar.activation(e[:], A[:], Exp, accum_out=Zt[:, q:q + 1])
                    nc.vector._custom_dve(
                        WTA_P2, out=B[:], in0=e[:], in1=A[:],
                        s0=mrec[:, s:s + 1], imm2=TH)
                    nc.vector.reciprocal(rt[:, q:q + 1], Zt[:, q:q + 1])
                    nc.vector.tensor_scalar(dt[:, j * F:(j + 1) * F], B[:],
                                            0.0, None, ALU.is_equal)
                nc.sync.dma_start(Dd[:, s0 * F:s1 * F], dt[:])
            nc.sync.dma_start(Md[:], mrec[:])
    nc.finalize()
    return nc


_LAYER_RESULTS_NS = []


def _run_layer(Ws, F, TH, S, Pc, trace=False):
    nc = _build_layer(Pc, F, S, TH)
    in_maps = [{"W": w} for w in Ws]
    res = bass_utils.run_bass_kernel_spmd(
        nc, in_maps, core_ids=list(range(N_CORES)), trace=trace)
    _LAYER_RESULTS_NS.append(res.exec_time_ns)
    return res.results


def kernel(x, w1, w2, w3, _trace=False):
    _LAYER_RESULTS_NS.clear()
    s = np.asarray(x, F32)
    for w, cfg in zip((w1, w2, w3), LAYERS):
        W_seq, T_seq, S, oh, ow = _build_events(s, np.asarray(w, F32), cfg['pad'])
        Ws, Pc = _shard(W_seq)
        F, TH = cfg['cout'], cfg['th']
        outs = _run_layer(Ws, F, TH, S, Pc, trace=_trace)
        L = oh * ow
        mrec = np.concatenate([r["m"] for r in outs], axis=0)[:L]        # (L, S)
        dd = np.concatenate([r["dd"] for r in outs], axis=0)[:L]         # (L, S*F)
        tg = np.where(mrec > TH, T_seq, 0.0).astype(F32)                 # (L, S)
        spk = (dd.reshape(L, S, F).astype(F32) * tg[:, :, None]).max(axis=1)
        s = _max_pool2(np.ascontiguousarray(spk.T.reshape(F, oh, ow)))
    return np.ascontiguousarray(s)


# revision 7
# speedup vs baseline: 2.2792x; 1.0492x over previous
"""CSNN (spiking conv net with WTA dynamics) on 8 Trainium2 NeuronCores.

The WTA event scan vectorizes across output columns (SBUF partitions);
output channels ride the free dim. Host pre-sorts each column's events by
spike time and pre-gathers weight rows into a (P, S*F) stream (zero rows pad
short columns; they add 0, never fire, and are exact no-ops).

Device per event step (the serial critical path is 3 instructions):
  pass1 (custom DVE op): pre = potU * sel(m_prev > TH, rt_prev, 1) + w_s,
                         with a fused max-reduction -> m_s.
      potU holds the *unnormalized* committed state (the winner-zeroed exp
      values when the previous step fired, the raw potential otherwise);
      multiplying by rt = 1/Z here reproduces fl(e_i * rt) + w bit-exactly,
      so normalization never occupies its own instruction.
  ACT:  e = Exp(pre), accumulator -> Z (sum includes the winner, as in the
        reference softmax).
  pass2 (custom DVE op): potU' = sel(m > TH, (pre != m) * e, pre)
      i.e. on fire commit the unnormalized exps with every argmax tie zeroed
      (exp > 0 elsewhere), else keep pre.
  off-path: DVE reciprocal (1/Z, ~9ns), GPSIMD records dd = (potU' == 0)
      (the winner one-hot) into a streamed uint8 buffer.

The device returns per-step column maxima (m) and winner one-hots (dd); the
host reconstructs spike times spk[p,f] = max_s fire[p,s]*dd[p,s,f]*T[p,s]
(event times ascend, so max == last, matching the reference overwrite).
Layers are separate launches; columns are sharded 8 ways.
"""
import numpy as np

import concourse.bacc as bacc
import concourse.mybir as mybir
from concourse.tile import TileContext
from concourse import bass_utils
from concourse import dve_ops
from concourse.dve_spec import (Spec, Src0, Src1, C0, C1, C2, Zero, One, MaxNeg,
                                select, ne, maxx, lower as dve_lower, _has_src1,
                                AluOp as DALU)
from concourse.dve_uop import DveOpSpec

F32 = np.float32
BF32 = mybir.dt.float32
Exp = mybir.ActivationFunctionType.Exp
ALU = mybir.AluOpType

LAYERS = [
    dict(cout=30, k=5, pad=2, th=2.4),
    dict(cout=100, k=3, pad=1, th=1.0),
    dict(cout=200, k=3, pad=1, th=1.0),
]
N_CORES = 8


# ------------------------------------------------------- custom DVE ops

def _register_dve(name, spec):
    for op in dve_ops.OPS:
        if op.name == name:
            return op
    row = dve_ops._CUSTOM_DVE_ROW_BASE + len(dve_ops.OPS)
    assert row < 0x20, "custom-DVE row field overflow"
    shas = {}
    for ver in ("v3",):  # TRN2
        uops = dve_lower(spec, ver=ver)
        shas[ver] = DveOpSpec(name=name, opcode=row, uops=uops,
                              rd1_en=_has_src1(spec)).sha(ver)
    op = dve_ops.DveOp(name, spec, subdim=False, uops_sha=shas)
    dve_ops.OPS.append(op)
    dve_ops._SUB_OPCODE_FOR_NAME[name] = row
    return op


def _ref_p1(in0, in1, s0, s1, imm2):
    b = (in0 * np.where(s1 > imm2, s0, 1.0) + in1).astype(np.float32)
    return b, np.maximum(b.reshape(b.shape[0], -1).max(-1, keepdims=True), 0.0)


# pre = potU * (m_prev > TH ? rt_prev : 1) + w   ; accum_out = max(pre)
# (maxx(Src0, MaxNeg) is an identity pad so the hoisted select is read at
# stage >= 1 -- latch-init can't build it by stage 0.)
WTA_P1 = _register_dve("WTA_P1", Spec(
    body=maxx(Src0, MaxNeg) * select(C1 > C2, C0, One) + Src1,
    accum=DALU.MAX, accum_init=Zero, reference=_ref_p1))

# potU' = (m > TH) ? (pre != m) * e : pre
WTA_P2 = _register_dve("WTA_P2", Spec(
    body=select(C0 > C2, ne(Src1, C0) * Src0, Src1),
    reference=lambda in0, in1, s0, s1, imm2: np.where(
        s0 > imm2, (in1 != s0).astype(np.float32) * in0, in1)))


# ---------------------------------------------------------------- host side

def _unfold_buggy(x, k):
    C, H, W = x.shape
    oh, ow = H - k + 1, W - k + 1
    ih = np.arange(oh)[:, None] + np.arange(k)[None, :]
    iw = np.arange(ow)[:, None] + np.arange(k)[None, :]
    p = x[:, ih[:, None, :, None], iw[None, :, None, :]]
    unf = p.transpose(0, 3, 4, 1, 2).reshape(C * k * k, oh * ow)
    return unf.reshape(C, oh * ow, k * k), oh, ow


def _build_events(spk_in, weights, pad):
    cout, cin, k, _ = weights.shape
    x = np.pad(spk_in.astype(F32), ((0, 0), (pad, pad), (pad, pad)))
    x_trans, oh, ow = _unfold_buggy(x, k)
    L, k2 = oh * ow, k * k
    w_r = np.ascontiguousarray(weights.reshape(cout, cin * k2).T.astype(F32))
    tv = x_trans.transpose(1, 0, 2).reshape(L, cin * k2)
    order = np.argsort(np.where(tv != 0, tv, np.inf), axis=1, kind='stable')
    nvalid = (tv != 0).sum(axis=1)
    S = max(1, int(nvalid.max()))
    order = order[:, :S]
    tsort = np.take_along_axis(tv, order, axis=1)
    valid = np.arange(S)[None, :] < nvalid[:, None]
    W_seq = w_r[order]
    W_seq[~valid] = 0.0
    T_seq = np.where(valid, tsort, 0.0).astype(F32)
    return np.ascontiguousarray(W_seq), T_seq, S, oh, ow


def _shard(W_seq):
    L, S, F = W_seq.shape
    Pc = (L + N_CORES - 1) // N_CORES
    Wp = np.zeros((Pc * N_CORES, S, F), F32)
    Wp[:L] = W_seq
    Ws = [np.ascontiguousarray(Wp[i * Pc:(i + 1) * Pc].reshape(Pc, S * F))
          for i in range(N_CORES)]
    return Ws, Pc


def _max_pool2(x):
    C, H, W = x.shape
    oh, ow = H // 2, W // 2
    return x[:, :oh * 2, :ow * 2].reshape(C, oh, 2, ow, 2).max(axis=(2, 4))


# -------------------------------------------------------------- device side

def _build_layer(P, F, S, TH, CS=None):
    if CS is None:
        # 3 W bufs (f32) + 3 dd bufs (u8) per chunk step: 15*F bytes/partition
        CS = max(1, min(S, (160 * 1024) // (15 * F)))
    nc = bacc.Bacc("TRN2", target_bir_lowering=False, debug=False)
    Wd = nc.dram_tensor("W", (P, S * F), BF32, kind="ExternalInput")
    Md = nc.dram_tensor("m", (P, S), BF32, kind="ExternalOutput")
    Dd = nc.dram_tensor("dd", (P, S * F), mybir.dt.uint8, kind="ExternalOutput")

    with TileContext(nc) as tc:
        with (
            tc.tile_pool(name="state", bufs=1) as st,
            tc.tile_pool(name="wpool", bufs=3) as wp,
            tc.tile_pool(name="dpool", bufs=3) as dp,
        ):
            A = st.tile([P, F], BF32)      # pre-commit potential
            B = st.tile([P, F], BF32)      # committed (unnormalized) state
            e = st.tile([P, F], BF32)
            mrec = st.tile([P, S], BF32)
            mz = st.tile([P, 1], BF32)     # m_prev for step 0
            Zt = st.tile([P, 2], BF32)     # parity-buffered Z
            rt = st.tile([P, 2], BF32)     # parity-buffered 1/Z

            nc.vector.memset(B[:], 0.0)
            nc.vector.memset(mz[:], 0.0)
            nc.vector.memset(rt[:], 1.0)
            nc.vector.memset(Zt[:], 1.0)

            # dd for step s-1 is emitted between P1_s and P2_s so it runs in
            # the DVE's exp-wait gap (it must precede P2_s, which overwrites
            # B). Each chunk's dd DMA-out is emitted right after the last dd
            # write into that chunk's tile.
            pending = []   # deferred emissions: dd ts op, then maybe chunk dma

            def flush_pending():
                while pending:
                    pending.pop(0)()

            nchunks = (S + CS - 1) // CS
            for ci in range(nchunks):
                c0, c1 = ci * CS, min(S, ci * CS + CS)
                wt = wp.tile([P, (c1 - c0) * F], BF32, tag="w")
                dt = dp.tile([P, (c1 - c0) * F], mybir.dt.uint8, tag="d")
                nc.sync.dma_start(wt[:], Wd[:, c0 * F:c1 * F])
                for s in range(c0, c1):
                    q, j = s & 1, s - c0
                    mprev = mz[:, 0:1] if s == 0 else mrec[:, s - 1:s]
                    nc.vector._custom_dve(
                        WTA_P1, out=A[:], in0=B[:], in1=wt[:, j * F:(j + 1) * F],
                        s0=rt[:, 1 - q:2 - q], s1=mprev, imm2=TH,
                        accum_out=mrec[:, s:s + 1])
                    nc.scalar.activation(e[:], A[:], Exp, accum_out=Zt[:, q:q + 1])
                    flush_pending()
                    nc.vector._custom_dve(
                        WTA_P2, out=B[:], in0=e[:], in1=A[:],
                        s0=mrec[:, s:s + 1], imm2=TH)
                    nc.vector.reciprocal(rt[:, q:q + 1], Zt[:, q:q + 1])
                    pending.append(
                        lambda dts=dt[:, j * F:(j + 1) * F]:
                            nc.vector.tensor_scalar(dts, B[:], 0.0, None,
                                                    ALU.is_equal))
                    if s == c1 - 1:
                        pending.append(
                            lambda a=Dd[:, c0 * F:c1 * F], b=dt:
                                nc.sync.dma_start(a, b[:]))
            flush_pending()
            nc.sync.dma_start(Md[:], mrec[:])
    nc.finalize()
    return nc


_LAYER_RESULTS_NS = []


def _run_layer(Ws, F, TH, S, Pc, trace=False):
    nc = _build_layer(Pc, F, S, TH)
    in_maps = [{"W": w} for w in Ws]
    res = bass_utils.run_bass_kernel_spmd(
        nc, in_maps, core_ids=list(range(N_CORES)), trace=trace)
    _LAYER_RESULTS_NS.append(res.exec_time_ns)
    return res.results


def kernel(x, w1, w2, w3, _trace=False):
    _LAYER_RESULTS_NS.clear()
    s = np.asarray(x, F32)
    for w, cfg in zip((w1, w2, w3), LAYERS):
        W_seq, T_seq, S, oh, ow = _build_events(s, np.asarray(w, F32), cfg['pad'])
        Ws, Pc = _shard(W_seq)
        F, TH = cfg['cout'], cfg['th']
        outs = _run_layer(Ws, F, TH, S, Pc, trace=_trace)
        L = oh * ow
        mrec = np.concatenate([r["m"] for r in outs], axis=0)[:L]        # (L, S)
        dd = np.concatenate([r["dd"] for r in outs], axis=0)[:L]         # (L, S*F)
        tg = np.where(mrec > TH, T_seq, 0.0).astype(F32)                 # (L, S)
        spk = (dd.reshape(L, S, F).astype(F32) * tg[:, :, None]).max(axis=1)
        s = _max_pool2(np.ascontiguousarray(spk.T.reshape(F, oh, ow)))
    return np.ascontiguousarray(s)


# revision 13
# speedup vs baseline: 2.2793x; 1.0000x over previous
"""CSNN (spiking conv net with WTA dynamics) on 8 Trainium2 NeuronCores.

The WTA event scan vectorizes across output columns (SBUF partitions);
output channels ride the free dim. Host pre-sorts each column's events by
spike time and pre-gathers weight rows into a (P, S*F) stream (zero rows pad
short columns; they add 0, never fire, and are exact no-ops).

Device per event step (the serial critical path is 3 instructions):
  pass1 (custom DVE op): pre = potU * sel(m_prev > TH, rt_prev, 1) + w_s,
                         with a fused max-reduction -> m_s.
      potU holds the *unnormalized* committed state (the winner-zeroed exp
      values when the previous step fired, the raw potential otherwise);
      multiplying by rt = 1/Z here reproduces fl(e_i * rt) + w bit-exactly,
      so normalization never occupies its own instruction.
  ACT:  e = Exp(pre), accumulator -> Z (sum includes the winner, as in the
        reference softmax).
  pass2 (custom DVE op): potU' = sel(m > TH, (pre != m) * e, pre)
      i.e. on fire commit the unnormalized exps with every argmax tie zeroed
      (exp > 0 elsewhere), else keep pre.
  off-path: DVE reciprocal (1/Z, ~9ns), GPSIMD records dd = (potU' == 0)
      (the winner one-hot) into a streamed uint8 buffer.

The device returns per-step column maxima (m) and winner one-hots (dd); the
host reconstructs spike times spk[p,f] = max_s fire[p,s]*dd[p,s,f]*T[p,s]
(event times ascend, so max == last, matching the reference overwrite).
Layers are separate launches; columns are sharded 8 ways.
"""
import numpy as np

import concourse.bacc as bacc
import concourse.mybir as mybir
from concourse.tile import TileContext
from concourse import bass_utils
from concourse import dve_ops
from concourse.dve_spec import (Spec, Src0, Src1, C0, C1, C2, Zero, One, MaxNeg,
                                select, ne, maxx, lower as dve_lower, _has_src1,
                                AluOp as DALU)
from concourse.dve_uop import DveOpSpec

F32 = np.float32
BF32 = mybir.dt.float32
Exp = mybir.ActivationFunctionType.Exp
ALU = mybir.AluOpType

LAYERS = [
    dict(cout=30, k=5, pad=2, th=2.4),
    dict(cout=100, k=3, pad=1, th=1.0),
    dict(cout=200, k=3, pad=1, th=1.0),
]
N_CORES = 8


# ------------------------------------------------------- custom DVE ops

def _register_dve(name, spec):
    for op in dve_ops.OPS:
        if op.name == name:
            return op
    row = dve_ops._CUSTOM_DVE_ROW_BASE + len(dve_ops.OPS)
    assert row < 0x20, "custom-DVE row field overflow"
    shas = {}
    for ver in ("v3",):  # TRN2
        uops = dve_lower(spec, ver=ver)
        shas[ver] = DveOpSpec(name=name, opcode=row, uops=uops,
                              rd1_en=_has_src1(spec)).sha(ver)
    op = dve_ops.DveOp(name, spec, subdim=False, uops_sha=shas)
    dve_ops.OPS.append(op)
    dve_ops._SUB_OPCODE_FOR_NAME[name] = row
    return op


def _ref_p1(in0, in1, s0, s1, imm2):
    b = (in0 * np.where(s1 > imm2, s0, 1.0) + in1).astype(np.float32)
    return b, np.maximum(b.reshape(b.shape[0], -1).max(-1, keepdims=True), 0.0)


# pre = potU * (m_prev > TH ? rt_prev : 1) + w   ; accum_out = max(pre)
# (maxx(Src0, MaxNeg) is an identity pad so the hoisted select is read at
# stage >= 1 -- latch-init can't build it by stage 0.)
WTA_P1 = _register_dve("WTA_P1", Spec(
    body=maxx(Src0, MaxNeg) * select(C1 > C2, C0, One) + Src1,
    accum=DALU.MAX, accum_init=Zero, reference=_ref_p1))

# potU' = (m > TH) ? (pre != m) * e : pre
WTA_P2 = _register_dve("WTA_P2", Spec(
    body=select(C0 > C2, ne(Src1, C0) * Src0, Src1),
    reference=lambda in0, in1, s0, s1, imm2: np.where(
        s0 > imm2, (in1 != s0).astype(np.float32) * in0, in1)))


# ---------------------------------------------------------------- host side

def _unfold_buggy(x, k):
    C, H, W = x.shape
    oh, ow = H - k + 1, W - k + 1
    ih = np.arange(oh)[:, None] + np.arange(k)[None, :]
    iw = np.arange(ow)[:, None] + np.arange(k)[None, :]
    p = x[:, ih[:, None, :, None], iw[None, :, None, :]]
    unf = p.transpose(0, 3, 4, 1, 2).reshape(C * k * k, oh * ow)
    return unf.reshape(C, oh * ow, k * k), oh, ow


def _build_events(spk_in, weights, pad):
    cout, cin, k, _ = weights.shape
    x = np.pad(spk_in.astype(F32), ((0, 0), (pad, pad), (pad, pad)))
    x_trans, oh, ow = _unfold_buggy(x, k)
    L, k2 = oh * ow, k * k
    w_r = np.ascontiguousarray(weights.reshape(cout, cin * k2).T.astype(F32))
    tv = x_trans.transpose(1, 0, 2).reshape(L, cin * k2)
    order = np.argsort(np.where(tv != 0, tv, np.inf), axis=1, kind='stable')
    nvalid = (tv != 0).sum(axis=1)
    S = max(1, int(nvalid.max()))
    order = order[:, :S]
    tsort = np.take_along_axis(tv, order, axis=1)
    valid = np.arange(S)[None, :] < nvalid[:, None]
    W_seq = w_r[order]
    W_seq[~valid] = 0.0
    T_seq = np.where(valid, tsort, 0.0).astype(F32)
    return np.ascontiguousarray(W_seq), T_seq, S, oh, ow


def _shard(W_seq):
    L, S, F = W_seq.shape
    Pc = (L + N_CORES - 1) // N_CORES
    Wp = np.zeros((Pc * N_CORES, S, F), F32)
    Wp[:L] = W_seq
    Ws = [np.ascontiguousarray(Wp[i * Pc:(i + 1) * Pc].reshape(Pc, S * F))
          for i in range(N_CORES)]
    return Ws, Pc


def _max_pool2(x):
    C, H, W = x.shape
    oh, ow = H // 2, W // 2
    return x[:, :oh * 2, :ow * 2].reshape(C, oh, 2, ow, 2).max(axis=(2, 4))


# -------------------------------------------------------------- device side

def _build_layer(P, F, S, TH, CS=None, a_space="SBUF", e_space="SBUF"):
    if CS is None:
        # 3 W bufs (f32) + 3 dd bufs (u8) per chunk step: 15*F bytes/partition
        CS = max(1, min(S, (160 * 1024) // (15 * F)))
    nc = bacc.Bacc("TRN2", target_bir_lowering=False, debug=False)
    Wd = nc.dram_tensor("W", (P, S * F), BF32, kind="ExternalInput")
    Md = nc.dram_tensor("m", (P, S), BF32, kind="ExternalOutput")
    Dd = nc.dram_tensor("dd", (P, S * F), mybir.dt.uint8, kind="ExternalOutput")

    with TileContext(nc) as tc:
        with (
            tc.tile_pool(name="state", bufs=1) as st,
            tc.tile_pool(name="apool", bufs=1, space=a_space) as ap,
            tc.tile_pool(name="epool", bufs=1, space=e_space) as ep,
            tc.tile_pool(name="wpool", bufs=3) as wp,
            tc.tile_pool(name="dpool", bufs=3) as dp,
        ):
            A = ap.tile([P, F], BF32)      # pre-commit potential
            B = st.tile([P, F], BF32)      # committed (unnormalized) state
            e = ep.tile([P, F], BF32)
            mrec = st.tile([P, S], BF32)
            mz = st.tile([P, 1], BF32)     # m_prev for step 0
            Zt = st.tile([P, 2], BF32)     # parity-buffered Z
            rt = st.tile([P, 2], BF32)     # parity-buffered 1/Z

            nc.vector.memset(B[:], 0.0)
            nc.vector.memset(mz[:], 0.0)
            nc.vector.memset(rt[:], 1.0)
            nc.vector.memset(Zt[:], 1.0)

            # dd for step s-1 is emitted between P1_s and P2_s so it runs in
            # the DVE's exp-wait gap (it must precede P2_s, which overwrites
            # B). Each chunk's dd DMA-out is emitted right after the last dd
            # write into that chunk's tile.
            pending = []   # deferred emissions: dd ts op, then maybe chunk dma

            def flush_pending():
                while pending:
                    pending.pop(0)()

            nchunks = (S + CS - 1) // CS
            for ci in range(nchunks):
                c0, c1 = ci * CS, min(S, ci * CS + CS)
                wt = wp.tile([P, (c1 - c0) * F], BF32, tag="w")
                dt = dp.tile([P, (c1 - c0) * F], mybir.dt.uint8, tag="d")
                nc.sync.dma_start(wt[:], Wd[:, c0 * F:c1 * F])
                for s in range(c0, c1):
                    q, j = s & 1, s - c0
                    mprev = mz[:, 0:1] if s == 0 else mrec[:, s - 1:s]
                    nc.vector._custom_dve(
                        WTA_P1, out=A[:], in0=B[:], in1=wt[:, j * F:(j + 1) * F],
                        s0=rt[:, 1 - q:2 - q], s1=mprev, imm2=TH,
                        accum_out=mrec[:, s:s + 1])
                    nc.scalar.activation(e[:], A[:], Exp, accum_out=Zt[:, q:q + 1])
                    flush_pending()
                    nc.vector._custom_dve(
                        WTA_P2, out=B[:], in0=e[:], in1=A[:],
                        s0=mrec[:, s:s + 1], imm2=TH)
                    nc.vector.reciprocal(rt[:, q:q + 1], Zt[:, q:q + 1])
                    pending.append(
                        lambda dts=dt[:, j * F:(j + 1) * F]:
                            nc.vector.tensor_scalar(dts, B[:], 0.0, None,
                                                    ALU.is_equal))
                    if s == c1 - 1:
                        pending.append(
                            lambda a=Dd[:, c0 * F:c1 * F], b=dt:
                                nc.sync.dma_start(a, b[:]))
            flush_pending()
            nc.sync.dma_start(Md[:], mrec[:])
    nc.finalize()
    return nc


_LAYER_RESULTS_NS = []


def _run_layer(Ws, F, TH, S, Pc, trace=False):
    nc = _build_layer(Pc, F, S, TH)
    in_maps = [{"W": w} for w in Ws]
    res = bass_utils.run_bass_kernel_spmd(
        nc, in_maps, core_ids=list(range(N_CORES)), trace=trace)
    _LAYER_RESULTS_NS.append(res.exec_time_ns)
    return res.results


def kernel(x, w1, w2, w3, _trace=False):
    _LAYER_RESULTS_NS.clear()
    s = np.asarray(x, F32)
    for w, cfg in zip((w1, w2, w3), LAYERS):
        W_seq, T_seq, S, oh, ow = _build_events(s, np.asarray(w, F32), cfg['pad'])
        Ws, Pc = _shard(W_seq)
        F, TH = cfg['cout'], cfg['th']
        outs = _run_layer(Ws, F, TH, S, Pc, trace=_trace)
        L = oh * ow
        mrec = np.concatenate([r["m"] for r in outs], axis=0)[:L]        # (L, S)
        dd = np.concatenate([r["dd"] for r in outs], axis=0)[:L]         # (L, S*F)
        tg = np.where(mrec > TH, T_seq, 0.0).astype(F32)                 # (L, S)
        spk = (dd.reshape(L, S, F).astype(F32) * tg[:, :, None]).max(axis=1)
        s = _max_pool2(np.ascontiguousarray(spk.T.reshape(F, oh, ow)))
    return np.ascontiguousarray(s)
